# revision 41
# baseline (speedup 1.0000x reference)
"""MLA (multi-head latent attention) forward on 8 TRN2 NeuronCores.

Sharding: core = 4*b + g  (b = batch 0..1, g = head-group 0..3, 4 heads each).
Each core compresses its own 512-token window, RMS-normalizes the latents in
place (rsqrt folded into the latents before the gather), AllGathers ckv+kr+cq
in ONE collective within its batch group (one op = one rendezvous + no serial
stream gaps), decompresses its 4 heads, runs causal attention over the full
2048 tokens with 128-granular triangular slicing, and projects to a bf16
partial output.  Host sums the 4 partials per batch.

Attention is organized per (head, query-half) pass: key-block (kc) outer,
query start at 128*kc, scores in [128, <=1024] PSUM chunks with a 3-deep
buffer and PV emitted two chunks behind, so the score->exp->PV chain latency
stays off the PE critical path.  The softmax denominator comes from a
ones-column in V; the PV psum is staged to SBUF so its reciprocal
(exp(-ln(d)) on ACT) and normalize run off the psum critical path.
"""

import sys

sys.path.insert(0, "/opt/trn_rl_repo")

import numpy as np
import ml_dtypes

from concourse import bacc, bass, bass_isa, mybir, tile
from concourse.bass_utils import run_bass_kernel_spmd

# problem dims (hardcoded per contract)
B, S, D = 2, 2048, 2048
H = 16
NOPE, ROPE, VD = 64, 32, 64
QR, KVR = 768, 256
EPS = 1e-6
THETA = 10000.0

HG = 4  # heads per core
NCORES = 8
P = 128
W = 512  # own-token window
NW = S // W  # 4
QKD = NOPE + ROPE  # 96
HALF = 1024  # query half (psum-chunk limit)
CKR = KVR + ROPE  # 288 latent rows for k/v
NKC = S // P  # 16 key blocks
ALLR = CKR + QR  # 1056 gathered rows

BF = mybir.dt.bfloat16
F32 = mybir.dt.float32
NBF = ml_dtypes.bfloat16
MULT = mybir.AluOpType.mult
AFT = mybir.ActivationFunctionType

LAST_RESULT = None
_CACHE = {}


def _build_nc():
    nc = bacc.Bacc("TRN2", debug=False)
    with tile.TileContext(nc) as tc:
        with (
            tc.tile_pool(name="dram", bufs=1, space="DRAM") as dram,
            tc.tile_pool(name="wres", bufs=1) as wres,
            tc.tile_pool(name="lat", bufs=1) as lat,
            tc.tile_pool(name="xin", bufs=1) as xin,
            tc.tile_pool(name="stg", bufs=2) as stg,
            tc.tile_pool(name="sqa", bufs=2) as sqa,
            tc.tile_pool(name="row", bufs=2) as rowp,
            tc.tile_pool(name="pt", bufs=4) as ptp,
            tc.tile_pool(name="rbc", bufs=2) as rbcp,
            tc.tile_pool(name="ovs", bufs=3) as ovsp,
            tc.tile_pool(name="ost", bufs=2) as ostp,
            tc.tile_pool(name="psA", bufs=3, space="PSUM") as psA,
            tc.tile_pool(name="psB", bufs=1, space="PSUM") as psB,
        ):
            # ---------------- DRAM params ----------------
            # all big params arrive pre-shuffled partition-major [128, c, k]
            xTw = dram.tile([P, D // P, W], BF, kind="ExternalInput", name="xTw", uniquify=False)
            wcq = dram.tile([P, D // P, QR], BF, kind="ExternalInput", name="wcq", uniquify=False)
            wckvkr = dram.tile(
                [P, D // P, CKR], BF, kind="ExternalInput", name="wckvkr", uniquify=False
            )
            wq = dram.tile(
                [P, QR // P, HG * QKD], BF, kind="ExternalInput", name="wq", uniquify=False
            )
            wkv = dram.tile(
                [P, KVR // P, HG * (NOPE + VD)], BF, kind="ExternalInput", name="wkv",
                uniquify=False,
            )
            wproj = dram.tile(
                [P, (HG * VD) // P, D], BF, kind="ExternalInput", name="wproj",
                uniquify=False,
            )
            cropeq_d = dram.tile(
                [QKD, S], BF, kind="ExternalInput", name="cropeq", uniquify=False
            )
            cropew_d = dram.tile(
                [ROPE, W], BF, kind="ExternalInput", name="cropew", uniquify=False
            )
            mask_d = dram.tile(
                [P, P], BF, kind="ExternalInput", name="mask", uniquify=False
            )
            out_d = dram.tile([S, D], BF, kind="ExternalOutput", name="out", uniquify=False)

            # collective buffers: ckv+kr gather early, cq gather second
            cc_in = dram.tile([ALLR, W], BF, kind="Internal", name="cc_in", uniquify=False)
            cc_oa = dram.tile([NW, CKR, W], BF, kind="Internal", name="cc_oa", uniquify=False)
            cc_ob = dram.tile([NW, QR, W], BF, kind="Internal", name="cc_ob", uniquify=False)

            GROUPS = [[0, 1, 2, 3], [4, 5, 6, 7]]

            # ---------------- resident SBUF ----------------
            # x staging shares its slot with cqT (x dies before cqT fills)
            NC_ = D // P  # 16 contraction chunks
            x_sb = xin.tile([P, NC_, W], BF, tag="big")
            wckvkr_sb = wres.tile([P, NC_, CKR], BF, tag="wckvkr")
            wcq_sb = wres.tile([P, NC_, QR], BF, tag="wcq")
            wq_sb = wres.tile([P, QR // P, HG * QKD], BF, tag="wq")
            wkv_sb = wres.tile([P, KVR // P, HG * (NOPE + VD)], BF, tag="wkv")
            wproj_sb = wres.tile([P, (HG * VD) // P, D], BF, tag="wproj")
            cropeq_sb = wres.tile([QKD, S], BF, tag="cropeq")
            cropew_sb = wres.tile([ROPE, W], BF, tag="cropew")
            mask_sb = wres.tile([P, P], BF, tag="mask")

            # x + compression weights, interleaved 2-chunk DMAs (pipelined MMs)
            for c4 in range(NC_ // 4):
                cs = slice(4 * c4, 4 * c4 + 4)
                nc.sync.dma_start(out=x_sb[:, cs, :], in_=xTw[:, cs, :])
                nc.sync.dma_start(out=wckvkr_sb[:, cs, :], in_=wckvkr[:, cs, :])
            for c4 in range(NC_ // 4):
                cs = slice(4 * c4, 4 * c4 + 4)
                nc.sync.dma_start(out=wcq_sb[:, cs, :], in_=wcq[:, cs, :])
            # decompress/attention weights (scalar queue; needed later)
            nc.scalar.dma_start(out=wq_sb[:], in_=wq[:])
            nc.scalar.dma_start(out=wkv_sb[:], in_=wkv[:])
            nc.scalar.dma_start(out=cropeq_sb[:], in_=cropeq_d[:])
            nc.scalar.dma_start(out=cropew_sb[:], in_=cropew_d[:])
            nc.scalar.dma_start(out=mask_sb[:], in_=mask_d[:])
            nc.scalar.dma_start(out=wproj_sb[:], in_=wproj[:])

            # eps row for the rsqrt (bias APs must be [P,1] SBUF)
            cb = wres.tile([P, 2], F32, tag="cb")
            nc.vector.memset(cb[:, 0:1], EPS)

            # ---------------- compression (own window) ----------------
            ckv_n = stg.tile([P, KVR // P, W], BF, tag="ckvn", bufs=1)
            cq_n = stg.tile([P, QR // P, W], BF, tag="cqn", bufs=1)
            kr_n = stg.tile([ROPE, W], BF, tag="krn", bufs=1)

            def compress_group(n_m, w_sb, raw, acc):
                for m in range(n_m):
                    ps = psA.tile([P, W], F32, tag="ps")
                    for c in range(NC_):
                        nc.tensor.matmul(
                            ps[:],
                            w_sb[:, c, m * P : (m + 1) * P],
                            x_sb[:, c, :],
                            start=(c == 0),
                            stop=(c == NC_ - 1),
                        )
                    nc.vector.tensor_copy(out=raw[:, m, :], in_=ps[:])
                    sq = sqa.tile([P, W], BF, tag="sq")
                    nc.scalar.square(out=sq[:], in_=ps[:])
                    if m == 0:
                        nc.vector.tensor_copy(out=acc[:], in_=sq[:])
                    else:
                        nc.vector.tensor_add(out=acc[:], in0=acc[:], in1=sq[:])

            def rsqrt_bcast(acc, inv_n):
                red = sqa.tile([P, W], F32, tag="red")
                nc.gpsimd.partition_all_reduce(
                    red[:], acc[:], channels=P, reduce_op=bass_isa.ReduceOp.add
                )
                srow = rowp.tile([1, W], F32, tag="srow", bufs=1)
                nc.scalar.activation(
                    out=srow[:], in_=red[0:1, :], func=AFT.Sqrt,
                    bias=cb[0:1, 0:1], scale=inv_n,
                )
                rrow = rowp.tile([1, W], F32, tag="rrow", bufs=1)
                nc.vector.reciprocal(out=rrow[:], in_=srow[:])
                rbc = sqa.tile([P, W], F32, tag="rbc")
                nc.gpsimd.partition_broadcast(rbc[:], rrow[:])
                return rbc

            # --- ckv (2 m-tiles) + kr ---
            acc_kv = sqa.tile([P, W], F32, tag="acckv", bufs=1)
            compress_group(KVR // P, wckvkr_sb, ckv_n, acc_kv)
            pkr = psA.tile([ROPE, W], F32, tag="ps")
            for c in range(NC_):
                nc.tensor.matmul(
                    pkr[:],
                    wckvkr_sb[:, c, KVR:CKR],
                    x_sb[:, c, :],
                    start=(c == 0),
                    stop=(c == NC_ - 1),
                )
            nc.vector.tensor_tensor(out=kr_n[:], in0=pkr[:], in1=cropew_sb[:], op=MULT)
            nc.sync.dma_start(out=cc_in[KVR:CKR, :], in_=kr_n[:])
            rbc_kv = rsqrt_bcast(acc_kv, 1.0 / KVR)
            for m in range(KVR // P):
                nc.vector.tensor_tensor(
                    out=ckv_n[:, m, :], in0=ckv_n[:, m, :], in1=rbc_kv[:], op=MULT
                )
            nc.sync.dma_start(
                out=cc_in[0:KVR, :].rearrange("(m p) w -> p m w", p=P), in_=ckv_n[:]
            )
            nc.gpsimd.collective_compute(
                "AllGather",
                mybir.AluOpType.bypass,
                replica_groups=GROUPS,
                ins=[cc_in[0:CKR, :]],
                outs=[cc_oa[:]],
            )

            # --- cq (6 m-tiles) ---
            acc_q = sqa.tile([P, W], F32, tag="accq", bufs=1)
            compress_group(QR // P, wcq_sb, cq_n, acc_q)
            rbc_q = rsqrt_bcast(acc_q, 1.0 / QR)
            for m in range(QR // P):
                nc.vector.tensor_tensor(
                    out=cq_n[:, m, :], in0=cq_n[:, m, :], in1=rbc_q[:], op=MULT
                )
            nc.sync.dma_start(
                out=cc_in[CKR:ALLR, :].rearrange("(m p) w -> p m w", p=P), in_=cq_n[:]
            )
            nc.gpsimd.collective_compute(
                "AllGather",
                mybir.AluOpType.bypass,
                replica_groups=GROUPS,
                ins=[cc_in[CKR:ALLR, :]],
                outs=[cc_ob[:]],
            )

            # ---------------- latent tiles (full sequence) ----------------
            ckvT = lat.tile([P, KVR // P, S], BF, tag="ckvT")
            krT = lat.tile([ROPE, S], BF, tag="krT")
            cqT = xin.tile([P, QR // P, S], BF, tag="big")  # reuses x slot

            for w in range(NW):
                ws = slice(w * W, (w + 1) * W)
                nc.scalar.dma_start(
                    out=ckvT[:, :, ws],
                    in_=cc_oa[w, 0:KVR, :].rearrange("(m p) w -> p m w", p=P),
                )
                nc.scalar.dma_start(out=krT[:, ws], in_=cc_oa[w, KVR:CKR, :])
            for w in range(NW):
                ws = slice(w * W, (w + 1) * W)
                nc.sync.dma_start(
                    out=cqT[:, :, ws],
                    in_=cc_ob[w, :, :].rearrange("(m p) w -> p m w", p=P),
                )

            # ---------------- decompress ----------------
            kT = [lat.tile([QKD, S], BF, tag=f"kT{h}", name=f"kT{h}") for h in range(HG)]
            qT = [lat.tile([QKD, S], BF, tag=f"qT{h}", name=f"qT{h}") for h in range(HG)]
            vaug = lat.tile([P, NKC, HG, VD + 1], BF, tag="vaug")
            oT = lat.tile([P, 2, S], BF, tag="oT")
            nc.vector.memset(vaug[:, :, :, VD : VD + 1], 1.0)

            # k_nope per (head, half): psum [64, HALF]
            for h in range(HG):
                for q2 in range(S // HALF):
                    hs = slice(q2 * HALF, (q2 + 1) * HALF)
                    ps = psA.tile([NOPE, HALF], F32, tag="ps")
                    for r in range(KVR // P):
                        for s2 in range(2):
                            nc.tensor.matmul(
                                ps[:, s2 * W : (s2 + 1) * W],
                                wkv_sb[:, r, h * NOPE : (h + 1) * NOPE],
                                ckvT[:, r, q2 * HALF + s2 * W : q2 * HALF + (s2 + 1) * W],
                                start=(r == 0),
                                stop=(r == KVR // P - 1),
                            )
                    nc.vector.tensor_copy(out=kT[h][0:NOPE, hs], in_=ps[:])
                nc.vector.tensor_copy(out=kT[h][NOPE:QKD, :], in_=krT[:])

            # v token-major: psum [128 tokens, HG*VD]
            for ck in range(NKC):
                ps = psA.tile([P, HG * VD], F32, tag="ps")
                for r in range(KVR // P):
                    nc.tensor.matmul(
                        ps[:],
                        ckvT[:, r, ck * P : (ck + 1) * P],
                        wkv_sb[:, r, HG * NOPE : HG * (NOPE + VD)],
                        start=(r == 0),
                        stop=(r == KVR // P - 1),
                    )
                nc.vector.tensor_copy(
                    out=vaug[:, ck, :, 0:VD],
                    in_=ps[:].rearrange("p (h d) -> p h d", h=HG),
                )

            # q per (head, half): psum [96, HALF]; rope rows get cos at evict
            for h in range(HG):
                for q2 in range(S // HALF):
                    hs = slice(q2 * HALF, (q2 + 1) * HALF)
                    ps = psA.tile([QKD, HALF], F32, tag="ps")
                    for r in range(QR // P):
                        for s2 in range(2):
                            nc.tensor.matmul(
                                ps[:, s2 * W : (s2 + 1) * W],
                                wq_sb[:, r, h * QKD : (h + 1) * QKD],
                                cqT[:, r, q2 * HALF + s2 * W : q2 * HALF + (s2 + 1) * W],
                                start=(r == 0),
                                stop=(r == QR // P - 1),
                            )
                    nc.vector.tensor_tensor(
                        out=qT[h][:, hs], in0=ps[:], in1=cropeq_sb[:, hs], op=MULT
                    )

            # ---------------- attention ----------------
            # per (head, query-half) pass: kc outer, query start at 128*kc.
            # single opv psum per pass (psB); scores triple-buffered (psA);
            # PV lags two chunks so the score->exp->(mask)->PV chain latency
            # hides under pipelined PE work.
            ostages = []

            def pieces(q0, q1):
                out = []
                a = q0
                while a < q1:
                    b = min((a // W + 1) * W, q1)
                    out.append((a, b))
                    a = b
                return out

            for h in range(HG):
                ostage = []
                for qh in range(2):
                    opv = psB.tile(
                        [VD + 1, HALF], F32, tag="opv", name=f"opv{h}_{qh}"
                    )
                    last_kc = (qh + 1) * (NKC // 2) - 1
                    pending = []

                    def flush_pv(n, opv=opv, qh=qh, h=h, last_kc=last_kc, pending=pending):
                        while len(pending) > n:
                            kc_, q0_, q1_, pt_, masked = pending.pop(0)
                            pcs = pieces(q0_, q1_)
                            if masked:  # emit non-masked pieces first
                                pcs = pcs[1:] + pcs[:1]
                            for a, b in pcs:
                                nc.tensor.matmul(
                                    opv[:, a - qh * HALF : b - qh * HALF],
                                    vaug[:, kc_, h, :],
                                    pt_[:, a - q0_ : b - q0_],
                                    start=(kc_ == 0),
                                    stop=(kc_ == last_kc),
                                )

                    for kc in range(last_kc + 1):
                        q0 = max(P * kc, qh * HALF)
                        q1 = (qh + 1) * HALF
                        sp = psA.tile([P, HALF], F32, tag="ps")
                        for a, b in pieces(q0, q1):
                            nc.tensor.matmul(
                                sp[:, a - qh * HALF : b - qh * HALF],
                                kT[h][:, kc * P : (kc + 1) * P],
                                qT[h][:, a:b],
                                start=True,
                                stop=True,
                            )
                        pt = ptp.tile([P, HALF], BF, tag="pt")
                        nc.scalar.activation(
                            out=pt[:, 0 : q1 - q0],
                            in_=sp[:, q0 - qh * HALF : HALF],
                            func=AFT.Exp,
                        )
                        masked = q0 == P * kc
                        if masked:  # diagonal chunk: mask first 128 cols
                            nc.vector.tensor_tensor(
                                out=pt[:, 0:P], in0=pt[:, 0:P], in1=mask_sb[:], op=MULT
                            )
                        pending.append((kc, q0, q1, pt, masked))
                        flush_pv(2)
                    flush_pv(0)

                    # stage the pass psum out so the denominator/normalize
                    # chain runs off the psum critical path
                    ov = ovsp.tile(
                        [VD + 1, HALF], BF, tag="ovs", name=f"ov{h}_{qh}"
                    )
                    nc.vector.tensor_copy(out=ov[:], in_=opv[:])
                    ostage.append(ov)
                ostages.append(ostage)

                # normalize: rec = exp(-ln(denom)); Lns then Exps batched per
                # head so the ACT table switches twice per head, not four times
                lnrs = []
                for qh in range(2):
                    lnr = rowp.tile([1, HALF], F32, tag="lnr", name=f"lnr{h}_{qh}")
                    nc.scalar.activation(
                        out=lnr[:], in_=ostage[qh][VD : VD + 1, :], func=AFT.Ln
                    )
                    lnrs.append(lnr)
                for qh in range(2):
                    rec = rowp.tile([1, HALF], BF, tag="rec", name=f"rec{h}_{qh}")
                    nc.scalar.activation(
                        out=rec[:], in_=lnrs[qh][:], func=AFT.Exp, scale=-1.0
                    )
                    rbc = rbcp.tile([VD, HALF], BF, tag="rbc")
                    nc.gpsimd.partition_broadcast(rbc[:], rec[:])
                    oTh = oT[VD * (h % 2) : VD * (h % 2) + VD, h // 2, :]
                    nc.vector.tensor_tensor(
                        out=oTh[:, qh * HALF : (qh + 1) * HALF],
                        in0=ostage[qh][0:VD, :],
                        in1=rbc[:],
                        op=MULT,
                    )

            # ---------------- projection ----------------
            for t in range(S // P):
                pps = []
                for wc in range(2):
                    pp = psA.tile([P, HALF], F32, tag="ps", name=f"pp{t}_{wc}")
                    for i in range(2):
                        for s2 in range(2):
                            nc.tensor.matmul(
                                pp[:, s2 * W : (s2 + 1) * W],
                                oT[:, i, t * P : (t + 1) * P],
                                wproj_sb[:, i, wc * HALF + s2 * W : wc * HALF + (s2 + 1) * W],
                                start=(i == 0),
                                stop=(i == 1),
                            )
                    pps.append(pp)
                for wc, pp in enumerate(pps):
                    o = ostp.tile([P, HALF], BF, tag="ost")
                    if wc == 0:
                        nc.vector.tensor_copy(out=o[:], in_=pp[:])
                    else:
                        nc.scalar.copy(out=o[:], in_=pp[:])
                    nc.sync.dma_start(
                        out=out_d[t * P : (t + 1) * P, wc * HALF : (wc + 1) * HALF],
                        in_=o[:],
                    )

    nc.compile()
    return nc


def _pmajor(a):
    """[C*128, K] row-major -> [128, C, K] partition-major bf16."""
    C = a.shape[0] // P
    return np.ascontiguousarray(
        a.reshape(C, P, a.shape[1]).transpose(1, 0, 2)
    ).astype(NBF)


def _rope_fold():
    """32x32 butterfly for RoPE with the reference's sin==cos bug."""
    Bm = np.zeros((ROPE, ROPE), np.float32)
    for j in range(ROPE // 2):
        Bm[2 * j, 2 * j] = 1.0
        Bm[2 * j, 2 * j + 1] = -1.0
        Bm[2 * j + 1, 2 * j] = 1.0
        Bm[2 * j + 1, 2 * j + 1] = 1.0
    return Bm


def _host_tables():
    freqs = 1.0 / (THETA ** (np.arange(0, ROPE, 2, dtype=np.float32) / ROPE))
    ang = np.outer(np.arange(S, dtype=np.float32), freqs)  # [S, 16]
    cos = np.cos(ang)
    crope32 = np.repeat(cos, 2, axis=1).T.copy().astype(np.float32)  # [32, S]
    cropeq = np.concatenate([np.ones((NOPE, S), np.float32), crope32], axis=0)
    mask = np.zeros((P, P), np.float32)
    for k in range(P):
        mask[k, k:] = 1.0
    return cropeq.astype(NBF), crope32.astype(NBF), mask.astype(NBF)


def kernel(**inputs):
    global LAST_RESULT
    x = np.asarray(inputs["x"], np.float32)
    w_cq = np.asarray(inputs["w_cq"], np.float32)
    w_q_nope = np.asarray(inputs["w_q_nope"], np.float32)
    w_q_rope = np.asarray(inputs["w_q_rope"], np.float32)
    q_g = np.asarray(inputs["q_g"], np.float32)
    w_ckv = np.asarray(inputs["w_ckv"], np.float32)
    w_k_nope = np.asarray(inputs["w_k_nope"], np.float32)
    w_v = np.asarray(inputs["w_v"], np.float32)
    kv_g = np.asarray(inputs["kv_g"], np.float32)
    w_k_rope = np.asarray(inputs["w_k_rope"], np.float32)
    w_proj = np.asarray(inputs["w_proj"], np.float32)

    Bm = _rope_fold()
    cropeq, crope32, mask = _host_tables()
    scale = 1.0 / np.sqrt(QKD)

    wqn = w_q_nope * q_g[:, None] * scale  # [QR, H*64]
    wqr = w_q_rope * q_g[:, None] * scale  # [QR, H*32]
    wkn = w_k_nope * kv_g[:, None]  # [KVR, H*64]
    wv = w_v * kv_g[:, None]  # [KVR, H*64]
    wkr = (w_k_rope @ Bm.T) / H  # [D, 32]
    wckvkr = np.concatenate([w_ckv, wkr], axis=1)  # [D, 288]

    if "nc" not in _CACHE:
        _CACHE["nc"] = _build_nc()
    nc = _CACHE["nc"]
    _PM = {"wcq": _pmajor(w_cq), "wckvkr": _pmajor(wckvkr)}

    in_maps = []
    for core in range(NCORES):
        b, g = divmod(core, NCORES // B)
        heads = range(HG * g, HG * (g + 1))
        wq_cols = []
        for h in heads:
            wq_cols.append(wqn[:, h * NOPE : (h + 1) * NOPE])
            wq_cols.append(wqr[:, h * ROPE : (h + 1) * ROPE] @ Bm.T)
        wq_core = np.concatenate(wq_cols, axis=1)  # [QR, 384]
        wkv_core = np.concatenate(
            [wkn[:, h * NOPE : (h + 1) * NOPE] for h in heads]
            + [wv[:, h * VD : (h + 1) * VD] for h in heads],
            axis=1,
        )  # [KVR, 512]
        wproj_core = np.concatenate(
            [w_proj[h * VD : (h + 1) * VD, :] for h in heads], axis=0
        )  # [256, D]
        in_maps.append(
            {
                "xTw": _pmajor(np.ascontiguousarray(x[b].T[:, W * g : W * (g + 1)])),
                "cropew": np.ascontiguousarray(crope32[:, W * g : W * (g + 1)]).astype(NBF),
                "wcq": _PM["wcq"],
                "wckvkr": _PM["wckvkr"],
                "wq": _pmajor(wq_core),
                "wkv": _pmajor(wkv_core),
                "wproj": _pmajor(wproj_core),
                "cropeq": cropeq,
                "mask": mask,
            }
        )

    res = run_bass_kernel_spmd(nc, in_maps, list(range(NCORES)))
    LAST_RESULT = res
    outs = [np.asarray(r["out"], np.float32) for r in res.results]
    gpb = NCORES // B
    out = np.stack(
        [sum(outs[b * gpb + g] for g in range(gpb)) for b in range(B)], axis=0
    )
    return out
